# revision 24
# baseline (speedup 1.0000x reference)
"""Trainium2 Bass kernel for AnchorProcessor (nms_detection).

Input  x: [8, 255, 128, 128] f32.  Output: [8, 18, 128, 128] f32.
Strategy: shard along H across 8 cores (16 rows each). Each core's problem is
fully local (the buggy cross-batch max/argmax reduces over (N, cls) which are
both on-core), so there are no collectives.

Per core (N=8, A=3, cls=80, HL=16, W=128):
  - box path (channels 0..3 per anchor) in natural layout:
      bx = sigmoid(tx) + gx, by = sigmoid(ty) + gy (grids passed as inputs),
      bw = tw * aw, bh = th * ah (anchor consts passed as inputs).
  - score path: for each anchor a and each h-row chunk j (128 pixels):
      PE-transpose the 80 logit channels of each n into PSUM [128pix x 80c],
      then one fused DVE tensor_tensor_reduce computes
      score = logitT * obj (broadcast) and max over the 640 flat (n,c) values,
      then max_index recovers the exact argmax (flat index n*80+c).
  - smax/sarg are transposed back ([128 x 16] -> [16 x 128]) and broadcast
    to all 8 batch entries in the output.
"""

import os
import sys

for _p in ("/opt/trn_rl_repo", "/root/.axon_site/_ro/trn_rl_repo"):
    if _p not in sys.path:
        sys.path.append(_p)

import numpy as np

from concourse import bacc, masks, mybir
from concourse.tile import TileContext

N = 8          # batch
A = 3          # anchors
CLS = 80       # classes per anchor
W = 128        # width
HL = 16        # local H rows per core (128 / 8 cores)
NCORES = 8

ANCHOR_W = (116.0, 156.0, 373.0)
ANCHOR_H = (90.0, 198.0, 326.0)

F32 = mybir.dt.float32
U32 = mybir.dt.uint32
NEG_INF = -3.0e38


def build_nc(hl=HL, reps=1, gp_num=3, gp_den=4, parts="full", argmax_eq=True):
    """Build the single-core graph (same SPMD graph on all 8 cores).

    reps > 1 repeats the whole computation on-device (for timing): the
    steady-state per-iteration time is (T(reps) - T(1)) / (reps - 1).

    gp_num/gp_den: fraction of chunks whose score-multiply runs on ACT
    (8 per-n scaled copies) instead of DVE tensor_mul - balances DVE vs ACT.
    argmax_eq: use the DVE scalar_tensor_tensor eq*iota-sum trick for the
    argmax instead of max_index.
    """
    import contextlib
    pix = hl * W           # pixels per core
    ch = hl                # one chunk per local h-row (128 pixels each)

    nc = bacc.Bacc("TRN2", target_bir_lowering=False, debug=False)

    x = nc.declare_dram_parameter("x", [N, 255, hl, W], F32, isOutput=False)
    grid = nc.declare_dram_parameter("grid", [2, A * N, pix], F32, isOutput=False)
    anch = nc.declare_dram_parameter("anch", [2, A * N, 1], F32, isOutput=False)
    iota = nc.declare_dram_parameter("iota", [N * CLS], F32, isOutput=False)
    out = nc.declare_dram_parameter("out", [N, A * 6, hl, W], F32, isOutput=True)

    with TileContext(nc) as tc:
        with (
            tc.tile_pool(name="const", bufs=1) as constp,
            tc.tile_pool(name="box", bufs=1) as boxp,
            tc.tile_pool(name="objsb", bufs=1) as objsbp,
            tc.tile_pool(name="lg", bufs=4) as lgp,
            tc.tile_pool(name="score", bufs=3) as scorep,
            tc.tile_pool(name="res", bufs=2) as resp,
            tc.tile_pool(name="outsb", bufs=3) as outsbp,
            tc.tile_pool(name="ps", bufs=3, space="PSUM") as psp,
            tc.tile_pool(name="ps2", bufs=1, space="PSUM") as ps2p,
        ):
            ident = constp.tile([128, 128], F32)
            masks.make_identity(nc, ident[:, :])

            gridt = [constp.tile([A * N, pix], F32, name=f"grid{g}", tag=f"grid{g}") for g in range(2)]
            ancht = [constp.tile([A * N, 1], F32, name=f"anch{g}", tag=f"anch{g}") for g in range(2)]
            for g in range(2):
                nc.sync.dma_start(out=gridt[g][:, :], in_=grid[g, :, :])
                nc.sync.dma_start(out=ancht[g][:, :], in_=anch[g, :, :])

            iotat = constp.tile([128, N * CLS], F32)
            nc.sync.dma_start(
                out=iotat[:, :],
                in_=iota[:].unsqueeze(0).broadcast_to([128, N * CLS]),
            )

            loop_cm = (
                tc.For_i(0, reps, 1, hint_engines=(mybir.EngineType.PE,))
                if reps > 1 else contextlib.nullcontext()
            )
            with loop_cm:
                body(nc, tc, x, out, pix, ch, hl,
                     ident, gridt, ancht, iotat, gp_num, gp_den, parts,
                     argmax_eq,
                     constp, boxp, objsbp, lgp, scorep, resp, outsbp, psp, ps2p)

    nc.compile()
    return nc


def body(nc, tc, x, out, pix, ch, hl, ident, gridt, ancht, iotat, gp_num, gp_den,
         parts, argmax_eq,
         constp, boxp, objsbp, lgp, scorep, resp, outsbp, psp, ps2p):
    if True:
        if True:
            # objectness planes, rows ordered (a, n) a-major
            objt = boxp.tile([A * N, pix], F32)
            nc.sync.dma_start(
                out=objt[:, :],
                in_=x[:, 4:255:85, :, :].transpose([1, 0, 2, 3]).rearrange(
                    "a n h w -> (a n) (h w)", keep_layout=True)
                if False else
                x[:, 4:255:85, :, :].transpose([1, 0, 2, 3]),
            )

            # parts: box | trans | objtr | full
            score_on = parts != "box"
            obj_tr_on = parts in ("full", "objtr")
            mult_mode = "copy_reduce" if parts in ("trans", "objtr") else "mult"
            argmax_on = parts == "full"

            # transposed objectness: objT[pix, chunk, a, n]
            objT = objsbp.tile([128, ch, A, N], F32)
            for j in range(ch if obj_tr_on else 0):
                ops = ps2p.tile([128, A * N], F32)
                nc.tensor.transpose(
                    ops[:, :], objt[:, j * 128:(j + 1) * 128], ident[:A * N, :A * N]
                )
                nc.scalar.copy(objT[:, j, :, :], ops[:, :])

            # ---------------- box path (natural layout) ----------------
            for k, name in ((0, "tx"), (1, "ty"), (2, "tw"), (3, "th")):
                t = boxp.tile([A * N, pix], F32, tag=f"box{k}")
                nc.sync.dma_start(
                    out=t[:, :],
                    in_=x[:, k:255:85, :, :].transpose([1, 0, 2, 3]),
                )
                o = boxp.tile([A * N, pix], F32, tag=f"boxo{k}")
                if k < 2:
                    nc.scalar.activation(
                        o[:, :], t[:, :], mybir.ActivationFunctionType.Sigmoid
                    )
                    # + gx (rows 0..23) or + gy (rows 24..47)
                    nc.vector.tensor_add(o[:, :], o[:, :], gridt[k][:, :])
                else:
                    # per-partition anchor const via ACT scale (keeps DVE free)
                    nc.scalar.mul(o[:, :], t[:, :], ancht[k - 2][:, :])
                nc.sync.dma_start(
                    out=out[:, k:18:6, :, :].transpose([1, 0, 2, 3]),
                    in_=o[:, :],
                )

            # ---------------- score path ----------------
            for a in range(A if score_on else 0):
                smaxT = resp.tile([128, ch], F32, tag="smaxT")
                sargT = resp.tile([128, ch], F32, tag="sargT")
                sargC = resp.tile([128, ch, 8], U32, tag="sargC")
                for j in range(ch):
                    # gather chunk j (h-row j) logits of all n: [80c, n, w]
                    lg = lgp.tile([80, N, W], F32)
                    nc.sync.dma_start(
                        out=lg[:, :, :],
                        in_=x[:, a * 85 + 5:a * 85 + 85, j, :].transpose([1, 0, 2]),
                    )
                    # transpose each n into PSUM: lgps[pix, n, c]
                    # (issue order alternates PSUM banks)
                    lgps = psp.tile([128, N, 128], F32)
                    for n in (0, 4, 1, 5, 2, 6, 3, 7):
                        nc.tensor.transpose(
                            lgps[:, n, 0:80], lg[:, n, :], ident[:80, :80]
                        )
                    score = scorep.tile([128, N, CLS], F32)
                    obj_b = objT[:, j, a, :].unsqueeze(2).broadcast_to(
                        [128, N, CLS])
                    if mult_mode == "copy_reduce":
                        nc.scalar.copy(score[:, :, :], lgps[:, :, 0:80])
                        nc.vector.reduce_max(
                            smaxT[:, j:j + 1],
                            score[:, :, :].rearrange("p n c -> p (n c)"),
                            axis=mybir.AxisListType.X,
                        )
                    elif (a * ch + j) % gp_den < gp_num:
                        # fused path: ACT evacuates PSUM, DVE does
                        # multiply+max in one tensor_tensor_reduce pass
                        ls = scorep.tile([128, N, CLS], F32, tag="ls")
                        nc.scalar.copy(ls[:, :, :], lgps[:, :, 0:80])
                        nc.vector.tensor_tensor_reduce(
                            out=score[:, :, :],
                            in0=ls[:, :, :],
                            in1=obj_b,
                            scale=1.0,
                            scalar=NEG_INF,
                            op0=mybir.AluOpType.mult,
                            op1=mybir.AluOpType.max,
                            accum_out=smaxT[:, j:j + 1],
                            opt_aps=False,
                        )
                    else:
                        # split path: DVE multiply (PSUM src), then DVE max
                        nc.vector.tensor_mul(score[:, :, :], lgps[:, :, 0:80], obj_b)
                        nc.vector.reduce_max(
                            smaxT[:, j:j + 1],
                            score[:, :, :].rearrange("p n c -> p (n c)"),
                            axis=mybir.AxisListType.X,
                        )
                    if not argmax_on:
                        continue
                    if argmax_eq:
                        # DVE argmax trick: sarg = sum((score == smax) * iota)
                        eqs = scorep.tile([128, N, CLS], F32, tag="eqs")
                        nc.vector.scalar_tensor_tensor(
                            out=eqs[:, :, :],
                            in0=score[:, :, :],
                            scalar=smaxT[:, j:j + 1],
                            in1=iotat[:, :].rearrange("p (n c) -> p n c", n=N),
                            op0=mybir.AluOpType.is_equal,
                            op1=mybir.AluOpType.mult,
                            accum_out=sargT[:, j:j + 1],
                        )
                    else:
                        # DVE argmax via max_index (uint32), convert to f32
                        nc.vector.max_index(
                            sargC[:, j, :],
                            smaxT[:, j:j + 1].broadcast_to([128, 8]),
                            score[:, :, :].rearrange("p n c -> p (n c)"),
                        )
                        nc.vector.tensor_copy(
                            sargT[:, j:j + 1], sargC[:, j, 0:1]
                        )

                outs_list = [(smaxT, a * 6 + 4)]
                if argmax_on:
                    outs_list.append((sargT, a * 6 + 5))
                for t_in, ch_out in outs_list:
                    tps = ps2p.tile([hl, 128], F32, tag="outps")
                    nc.tensor.transpose(tps[:, :], t_in[:, :], ident[:, :])
                    osb = outsbp.tile([hl, 128], F32, tag="osb")
                    nc.scalar.copy(osb[:, :], tps[:, :])
                    for n in range(N):
                        nc.sync.dma_start(
                            out=out[n, ch_out, :, :], in_=osb[:, :]
                        )


_NC_CACHE = {}


def get_nc(hl=HL):
    if hl not in _NC_CACHE:
        _NC_CACHE[hl] = build_nc(hl)
    return _NC_CACHE[hl]


def make_in_maps(x, hl=HL):
    """Shard the full input along H and build per-core input maps."""
    x = np.ascontiguousarray(x, dtype=np.float32)
    pix = hl * W
    gx = np.tile(np.arange(W, dtype=np.float32), hl)          # value = w
    anch_col = np.stack(
        [np.repeat(np.array(ANCHOR_W, np.float32), N),
         np.repeat(np.array(ANCHOR_H, np.float32), N)]
    ).reshape(2, A * N, 1)
    in_maps = []
    ncores = x.shape[2] // hl
    for i in range(ncores):
        gy = np.repeat(np.arange(i * hl, (i + 1) * hl, dtype=np.float32), W)
        grid = np.empty((2, A * N, pix), np.float32)
        grid[0] = gx
        grid[1] = gy
        in_maps.append({
            "x": np.ascontiguousarray(x[:, :, i * hl:(i + 1) * hl, :]),
            "grid": grid,
            "anch": anch_col,
            "iota": np.arange(N * CLS, dtype=np.float32),
        })
    return in_maps


def patch_compile_cache(cache_dir="/tmp/bass_neff_cache"):
    """Cache compiled NEFFs on disk keyed by the BIR hash (compile takes
    minutes; the cache makes repeated runs of an identical graph instant)."""
    import hashlib
    import shutil
    import concourse.bass2jax as b2j

    if getattr(b2j, "_neff_cache_patched", False):
        return
    os.makedirs(cache_dir, exist_ok=True)
    orig = b2j.compile_bir_kernel

    def cached(bir_json, tmpdir, neff_name="file.neff"):
        data = bir_json if isinstance(bir_json, bytes) else str(bir_json).encode()
        key = hashlib.sha256(data).hexdigest()[:32]
        cpath = os.path.join(cache_dir, key + ".neff")
        if os.path.exists(cpath):
            opath = os.path.join(tmpdir, neff_name)
            shutil.copy(cpath, opath)
            return opath
        r = orig(bir_json, tmpdir, neff_name)
        try:
            shutil.copy(r, cpath)
        except OSError:
            pass
        return r

    b2j.compile_bir_kernel = cached
    b2j._neff_cache_patched = True


def kernel(x: np.ndarray) -> np.ndarray:
    from concourse.bass_utils import run_bass_kernel_spmd

    patch_compile_cache()

    nc = get_nc(HL)
    in_maps = make_in_maps(x, HL)
    res = run_bass_kernel_spmd(nc, in_maps, core_ids=list(range(NCORES)))
    return np.concatenate([res.results[i]["out"] for i in range(NCORES)], axis=2)
